# revision 6
# baseline (speedup 1.0000x reference)
"""CartesianMACE rank-0 fused kernel for 8 trn2 NeuronCores.

The reference's ranks 1 and 2 never reach the output (each rank is mixed
independently and the head reads only h[0]), so only the rank-0 slices of
cw0/mw0/cw1/mw1 plus h0/msg0_r0/msg1_r0/w_pred/b_pred are needed.

Per node n (16x16 mats A=cw0[0,n], B=mw0[0,n], D=cw1[0,n], E=mw1[0,n];
16-vecs x=h0[n], m0=msg0_r0[n], m1=msg1_r0[n]):
    s[n] = colsum(D) . (A x + B m0) + colsum(E) . m1
    out  = [sum_n s[n] w_pred[0,n], sum_n s[n] w_pred[1,n]] + b_pred

Sharding: data-parallel over nodes. 50000 nodes padded to 50176 =
8 cores x 7 supertiles x 128 partitions x 7 groups. Nodes live on SBUF
partitions; the 256-element flattened matrices live on the free axis.
All compute on the vector engine; per-core (128,2) partials are summed
on host (the final all-reduce of the head).
"""

import sys
import types

for _p in ("/opt/trn_rl_repo", "/root/.axon_site/_ro/trn_rl_repo"):
    if _p not in sys.path:
        sys.path.append(_p)

import numpy as np

N, CH = 50000, 16
CORES = 8
T, S = 7, 7          # supertiles per core, groups per supertile
GP = T * S           # 49 groups of 128 nodes per core
NP = CORES * T * 128 * S  # 50176 padded nodes

_cache = {}
TRACE = False  # set by test harness to capture an NTFF profile


def _split_multiwait(nc, mybir):
    """This walrus build accepts a single sync-wait per instruction, but Tile
    attaches one wait per producer proc. Split: keep the last wait on the
    instruction and hoist the rest onto fresh same-engine Drain carriers
    inserted immediately before it (engines execute their stream in-order,
    so semantics are identical)."""
    for fn in nc.m.functions:
        for bb in fn.blocks:
            insts = bb.instructions  # live list
            i = 0
            while i < len(insts):
                ins = insts[i]
                si = ins.sync_info
                if si is not None and len(si.on_wait) > 1:
                    waits = list(si.on_wait)
                    ins.sync_info = mybir.SyncInfo(
                        on_wait=waits[-1:], on_update=list(si.on_update))
                    for k, w in enumerate(waits[:-1]):
                        insts.insert(i + k, mybir.InstDrain(
                            name=f"{ins.name}_w{k}", opcode="Drain",
                            engine=ins.engine, ins=[], outs=[],
                            sync_info=mybir.SyncInfo(on_wait=[w], on_update=[]),
                        ))
                    i += len(waits) - 1
                i += 1


def _build_nc():
    import concourse.bass as bass
    import concourse.tile as tile
    import concourse.mybir as mybir

    f32 = mybir.dt.float32
    P = 128

    nc = bass.Bass("TRN2", target_bir_lowering=False, debug=False,
                   num_devices=CORES)

    a_d = nc.dram_tensor("a", [T, P, S * 256], f32, kind="ExternalInput").ap()
    b_d = nc.dram_tensor("b", [T, P, S * 256], f32, kind="ExternalInput").ap()
    d_d = nc.dram_tensor("d", [T, P, S * 256], f32, kind="ExternalInput").ap()
    e_d = nc.dram_tensor("e", [T, P, S * 256], f32, kind="ExternalInput").ap()
    x_d = nc.dram_tensor("x", [T, P, S * 16], f32, kind="ExternalInput").ap()
    m0_d = nc.dram_tensor("m0", [T, P, S * 16], f32, kind="ExternalInput").ap()
    m1_d = nc.dram_tensor("m1", [T, P, S * 16], f32, kind="ExternalInput").ap()
    w_d = nc.dram_tensor("w", [P, 2 * GP], f32, kind="ExternalInput").ap()
    o_d = nc.dram_tensor("o", [P, 2], f32, kind="ExternalOutput").ap()

    with tile.TileContext(nc) as tc:
        with (
            tc.tile_pool(name="mats", bufs=2) as mats,
            tc.tile_pool(name="vecs", bufs=2) as vecs,
            tc.tile_pool(name="work", bufs=2) as work,
            tc.tile_pool(name="acc", bufs=1) as acc,
        ):
            s_all = acc.tile([P, GP], f32)
            w_sb = acc.tile([P, 2 * GP], f32)
            nc.sync.dma_start(out=w_sb[:, :], in_=w_d)

            for t in range(T):
                a_sb = mats.tile([P, S * 256], f32, tag="a")
                nc.sync.dma_start(out=a_sb[:, :], in_=a_d[t])
                b_sb = mats.tile([P, S * 256], f32, tag="b")
                nc.sync.dma_start(out=b_sb[:, :], in_=b_d[t])
                d_sb = mats.tile([P, S * 256], f32, tag="d")
                nc.sync.dma_start(out=d_sb[:, :], in_=d_d[t])
                e_sb = mats.tile([P, S * 256], f32, tag="e")
                nc.sync.dma_start(out=e_sb[:, :], in_=e_d[t])
                x_sb = vecs.tile([P, S * 16], f32, tag="x")
                nc.sync.dma_start(out=x_sb[:, :], in_=x_d[t])
                m0_sb = vecs.tile([P, S * 16], f32, tag="m0")
                nc.sync.dma_start(out=m0_sb[:, :], in_=m0_d[t])
                m1_sb = vecs.tile([P, S * 16], f32, tag="m1")
                nc.sync.dma_start(out=m1_sb[:, :], in_=m1_d[t])

                gjk = lambda ap: ap.rearrange("p (g j k) -> p g j k",
                                              g=S, j=16, k=16)
                bc = lambda ap: (ap.rearrange("p (g k) -> p g k", g=S, k=16)
                                 .unsqueeze(2).broadcast_to((P, S, 16, 16)))

                # tmp[:, g, 0] = A * rep(x);  tmp[:, g, 1] = B * rep(m0)
                tmp = work.tile([P, S * 512], f32, tag="tmp")
                tmp4 = tmp[:, :].rearrange("p (g m j k) -> p g m j k",
                                           g=S, m=2, j=16, k=16)
                nc.vector.tensor_mul(out=tmp4[:, :, 0], in0=gjk(a_sb[:, :]),
                                     in1=bc(x_sb[:, :]))
                nc.vector.tensor_mul(out=tmp4[:, :, 1], in0=gjk(b_sb[:, :]),
                                     in1=bc(m0_sb[:, :]))

                # c = A*rep(x) + B*rep(m0); t[g,j] = sum_k c[g,j,k]
                c_sb = work.tile([P, S * 256], f32, tag="c")
                tmp3 = tmp[:, :].rearrange("p (g m e) -> p g m e",
                                           g=S, m=2, e=256)
                nc.vector.tensor_add(
                    out=c_sb[:, :].rearrange("p (g e) -> p g e", g=S, e=256),
                    in0=tmp3[:, :, 0], in1=tmp3[:, :, 1])
                tv = work.tile([P, S * 16], f32, tag="tv")
                nc.vector.reduce_sum(
                    out=tv[:, :].rearrange("p (g j) -> p g j", g=S, j=16),
                    in_=gjk(c_sb[:, :]), axis=mybir.AxisListType.X)

                # d[g,k] = sum_j D[g,j,k]; e[g,k] = sum_j E[g,j,k]
                de = work.tile([P, S * 32], f32, tag="de")
                de4 = de[:, :].rearrange("p (g h k) -> p g h k",
                                         g=S, h=2, k=16)
                nc.vector.reduce_sum(
                    out=de4[:, :, 0],
                    in_=d_sb[:, :].rearrange("p (g j k) -> p g k j",
                                             g=S, j=16, k=16),
                    axis=mybir.AxisListType.X)
                nc.vector.reduce_sum(
                    out=de4[:, :, 1],
                    in_=e_sb[:, :].rearrange("p (g j k) -> p g k j",
                                             g=S, j=16, k=16),
                    axis=mybir.AxisListType.X)

                # s[g] = d . t + e . m1
                pr = work.tile([P, S * 32], f32, tag="pr")
                pr4 = pr[:, :].rearrange("p (g h k) -> p g h k",
                                         g=S, h=2, k=16)
                nc.vector.tensor_mul(
                    out=pr4[:, :, 0], in0=de4[:, :, 0],
                    in1=tv[:, :].rearrange("p (g j) -> p g j", g=S, j=16))
                nc.vector.tensor_mul(
                    out=pr4[:, :, 1], in0=de4[:, :, 1],
                    in1=m1_sb[:, :].rearrange("p (g k) -> p g k", g=S, k=16))
                nc.vector.reduce_sum(
                    out=s_all[:, t * S:(t + 1) * S],
                    in_=pr[:, :].rearrange("p (g i) -> p g i", g=S, i=32),
                    axis=mybir.AxisListType.X)

            # head partials: o[:, c] = sum_g s_all[:, g] * w[:, c*GP+g]
            junk = acc.tile([P, GP], f32)
            o_sb = acc.tile([P, 2], f32)
            for chn in range(2):
                nc.vector.tensor_mul(
                    out=junk[:, :], in0=s_all[:, :],
                    in1=w_sb[:, chn * GP:(chn + 1) * GP])
                nc.vector.reduce_sum(
                    out=o_sb[:, chn:chn + 1], in_=junk[:, :],
                    axis=mybir.AxisListType.X)
            nc.sync.dma_start(out=o_d, in_=o_sb[:, :])

    return nc


def _get_nc():
    if "nc" not in _cache:
        _cache["nc"] = _build_nc()
    return _cache["nc"]


def _shard_mat(m):
    """(N,16,16) -> (CORES, T, 128, S*256), zero-padded, group-major free axis."""
    out = np.zeros((NP, 256), np.float32)
    out[:N] = np.asarray(m, np.float32).reshape(N, 256)
    return np.ascontiguousarray(out.reshape(CORES, T, 128, S * 256))


def _shard_vec(v):
    """(N,16) -> (CORES, T, 128, S*16)."""
    out = np.zeros((NP, 16), np.float32)
    out[:N] = np.asarray(v, np.float32).reshape(N, 16)
    return np.ascontiguousarray(out.reshape(CORES, T, 128, S * 16))


def kernel(h0, cw0, mw0, cw1, mw1,
           msg0_r0, msg0_r1, msg0_r2,
           msg1_r0, msg1_r1, msg1_r2,
           w_pred, b_pred):
    from concourse.bass_utils import run_bass_kernel_spmd

    nc = _get_nc()
    if not _cache.get("split_done"):
        import concourse.mybir as mybir
        _split_multiwait(nc, mybir)
        _cache["split_done"] = True

    A = _shard_mat(cw0[0])
    B = _shard_mat(mw0[0])
    D = _shard_mat(cw1[0])
    E = _shard_mat(mw1[0])
    X = _shard_vec(np.asarray(h0, np.float32)[..., 0])
    M0 = _shard_vec(np.asarray(msg0_r0, np.float32)[..., 0])
    M1 = _shard_vec(np.asarray(msg1_r0, np.float32)[..., 0])

    wp = np.zeros((2, NP), np.float32)
    wp[:, :N] = np.asarray(w_pred, np.float32)
    # (2, CORES, T, 128, S) -> (CORES, 128, 2, T, S) -> (CORES, 128, 2*GP)
    W = np.ascontiguousarray(
        wp.reshape(2, CORES, T, 128, S).transpose(1, 3, 0, 2, 4)
        .reshape(CORES, 128, 2 * GP))

    in_maps = [
        {"a": A[i], "b": B[i], "d": D[i], "e": E[i],
         "x": X[i], "m0": M0[i], "m1": M1[i], "w": W[i]}
        for i in range(CORES)
    ]
    res = run_bass_kernel_spmd(nc, in_maps, list(range(CORES)), trace=TRACE)
    _cache["last_res"] = res
    partial = np.zeros(2, np.float64)
    for i in range(CORES):
        partial += res.results[i]["o"].astype(np.float64).sum(axis=0)
    out = (partial + np.asarray(b_pred, np.float64)).astype(np.float32)
    return out.reshape(1, 2)


# revision 9
# speedup vs baseline: 1.1207x; 1.1207x over previous
"""CartesianMACE rank-0 fused kernel for 8 trn2 NeuronCores.

The reference's ranks 1 and 2 never reach the output (each rank is mixed
independently and the head reads only h[0]), so only the rank-0 slices of
cw0/mw0/cw1/mw1 plus h0/msg0_r0/msg1_r0/w_pred/b_pred are needed.

Per node n (16x16 mats A=cw0[0,n], B=mw0[0,n], D=cw1[0,n], E=mw1[0,n];
16-vecs x=h0[n], m0=msg0_r0[n], m1=msg1_r0[n]):
    s[n] = colsum(D) . (A x + B m0) + colsum(E) . m1
    out  = [sum_n s[n] w_pred[0,n], sum_n s[n] w_pred[1,n]] + b_pred

Sharding: data-parallel over nodes. 50000 nodes padded to 50176 =
8 cores x 7 supertiles x 128 partitions x 7 groups. Nodes live on SBUF
partitions; the 256-element flattened matrices live on the free axis.
All compute on the vector engine; per-core (128,2) partials are summed
on host (the final all-reduce of the head).
"""

import sys
import types

for _p in ("/opt/trn_rl_repo", "/root/.axon_site/_ro/trn_rl_repo"):
    if _p not in sys.path:
        sys.path.append(_p)

import numpy as np

N, CH = 50000, 16
CORES = 8
T, S = 7, 7          # supertiles per core, groups per supertile
GP = T * S           # 49 groups of 128 nodes per core
NP = CORES * T * 128 * S  # 50176 padded nodes

_cache = {}
TRACE = False  # set by test harness to capture an NTFF profile


def _split_multiwait(nc, mybir):
    """This walrus build accepts a single sync-wait per instruction, but Tile
    attaches one wait per producer proc. Split: keep the last wait on the
    instruction and hoist the rest onto fresh same-engine Drain carriers
    inserted immediately before it (engines execute their stream in-order,
    so semantics are identical)."""
    for fn in nc.m.functions:
        for bb in fn.blocks:
            insts = bb.instructions  # live list
            i = 0
            while i < len(insts):
                ins = insts[i]
                si = ins.sync_info
                if si is not None and len(si.on_wait) > 1:
                    waits = list(si.on_wait)
                    ins.sync_info = mybir.SyncInfo(
                        on_wait=waits[-1:], on_update=list(si.on_update))
                    for k, w in enumerate(waits[:-1]):
                        insts.insert(i + k, mybir.InstDrain(
                            name=f"{ins.name}_w{k}", opcode="Drain",
                            engine=ins.engine, ins=[], outs=[],
                            sync_info=mybir.SyncInfo(on_wait=[w], on_update=[]),
                        ))
                    i += len(waits) - 1
                i += 1


def _build_nc():
    import concourse.bass as bass
    import concourse.tile as tile
    import concourse.mybir as mybir

    f32 = mybir.dt.float32
    P = 128

    nc = bass.Bass("TRN2", target_bir_lowering=False, debug=False,
                   num_devices=CORES)

    a_d = nc.dram_tensor("a", [T, P, S * 256], f32, kind="ExternalInput").ap()
    b_d = nc.dram_tensor("b", [T, P, S * 256], f32, kind="ExternalInput").ap()
    de_d = nc.dram_tensor("de", [T, P, S * 512], f32,
                          kind="ExternalInput").ap()
    x_d = nc.dram_tensor("x", [T, P, S * 16], f32, kind="ExternalInput").ap()
    m0_d = nc.dram_tensor("m0", [T, P, S * 16], f32, kind="ExternalInput").ap()
    m1_d = nc.dram_tensor("m1", [T, P, S * 16], f32, kind="ExternalInput").ap()
    w_d = nc.dram_tensor("w", [P, 2 * GP], f32, kind="ExternalInput").ap()
    o_d = nc.dram_tensor("o", [P, 2], f32, kind="ExternalOutput").ap()

    with tile.TileContext(nc) as tc:
        with (
            tc.tile_pool(name="mats", bufs=2) as mats,
            tc.tile_pool(name="vecs", bufs=2) as vecs,
            tc.tile_pool(name="work", bufs=2) as work,
            tc.tile_pool(name="acc", bufs=1) as acc,
        ):
            s_all = acc.tile([P, GP], f32)
            w_sb = acc.tile([P, 2 * GP], f32)
            nc.sync.dma_start(out=w_sb[:, :], in_=w_d)

            for t in range(T):
                a_sb = mats.tile([P, S * 256], f32, tag="a")
                nc.sync.dma_start(out=a_sb[:, :], in_=a_d[t])
                b_sb = mats.tile([P, S * 256], f32, tag="b")
                nc.sync.dma_start(out=b_sb[:, :], in_=b_d[t])
                de_sb = mats.tile([P, S * 512], f32, tag="de")
                nc.sync.dma_start(out=de_sb[:, :], in_=de_d[t])
                x_sb = vecs.tile([P, S * 16], f32, tag="x")
                nc.sync.dma_start(out=x_sb[:, :], in_=x_d[t])
                m0_sb = vecs.tile([P, S * 16], f32, tag="m0")
                nc.sync.dma_start(out=m0_sb[:, :], in_=m0_d[t])
                m1_sb = vecs.tile([P, S * 16], f32, tag="m1")
                nc.sync.dma_start(out=m1_sb[:, :], in_=m1_d[t])

                gjk = lambda ap: ap.rearrange("p (g j k) -> p g j k",
                                              g=S, j=16, k=16)
                bc = lambda ap: (ap.rearrange("p (g k) -> p g k", g=S, k=16)
                                 .unsqueeze(2).broadcast_to((P, S, 16, 16)))

                # tmp[g, j, 0:16] = A row j * x; tmp[g, j, 16:32] = B row j * m0
                tmp = work.tile([P, S * 512], f32, tag="tmp")
                tmp4 = tmp[:, :].rearrange("p (g j m k) -> p g j m k",
                                           g=S, j=16, m=2, k=16)
                nc.vector.tensor_mul(out=tmp4[:, :, :, 0],
                                     in0=gjk(a_sb[:, :]), in1=bc(x_sb[:, :]))
                nc.vector.tensor_mul(out=tmp4[:, :, :, 1],
                                     in0=gjk(b_sb[:, :]), in1=bc(m0_sb[:, :]))

                # t[g,j] = sum over the 32-wide block (A and B contributions)
                tv = work.tile([P, S * 16], f32, tag="tv")
                nc.vector.reduce_sum(
                    out=tv[:, :].rearrange("p (g j) -> p g j", g=S, j=16),
                    in_=tmp[:, :].rearrange("p (g j i) -> p g j i",
                                            g=S, j=16, i=32),
                    axis=mybir.AxisListType.X)

                # colsums of D and E on GpSimd: interleaved halving chain
                # de_sb layout per group: D(256 j-major) | E(256 j-major)
                h1 = work.tile([P, S * 256], f32, tag="h1")
                h2 = work.tile([P, S * 128], f32, tag="h2")
                h3 = work.tile([P, S * 64], f32, tag="h3")
                deq = work.tile([P, S * 32], f32, tag="deq")

                def gmi(ap, w):
                    return ap.rearrange("p (g m i) -> p g m i", g=S, m=2, i=w)

                src = de_sb[:, :].rearrange("p (g m i) -> p g m i",
                                            g=S, m=2, i=256)
                dst = gmi(h1[:, :], 128)
                nc.gpsimd.tensor_add(out=dst, in0=src[:, :, :, 0:128],
                                     in1=src[:, :, :, 128:256])
                src = dst
                dst = gmi(h2[:, :], 64)
                nc.gpsimd.tensor_add(out=dst, in0=src[:, :, :, 0:64],
                                     in1=src[:, :, :, 64:128])
                src = dst
                dst = gmi(h3[:, :], 32)
                nc.gpsimd.tensor_add(out=dst, in0=src[:, :, :, 0:32],
                                     in1=src[:, :, :, 32:64])
                src = dst
                deq4 = gmi(deq[:, :], 16)
                nc.gpsimd.tensor_add(out=deq4, in0=src[:, :, :, 0:16],
                                     in1=src[:, :, :, 16:32])

                # s[g] = d . t + e . m1
                pr = work.tile([P, S * 32], f32, tag="pr")
                pr4 = gmi(pr[:, :], 16)
                nc.vector.tensor_mul(
                    out=pr4[:, :, 0], in0=deq4[:, :, 0],
                    in1=tv[:, :].rearrange("p (g j) -> p g j", g=S, j=16))
                nc.vector.tensor_mul(
                    out=pr4[:, :, 1], in0=deq4[:, :, 1],
                    in1=m1_sb[:, :].rearrange("p (g k) -> p g k", g=S, k=16))
                nc.vector.reduce_sum(
                    out=s_all[:, t * S:(t + 1) * S],
                    in_=pr[:, :].rearrange("p (g i) -> p g i", g=S, i=32),
                    axis=mybir.AxisListType.X)

            # head partials: o[:, c] = sum_g s_all[:, g] * w[:, c*GP+g]
            junk = acc.tile([P, GP], f32)
            o_sb = acc.tile([P, 2], f32)
            for chn in range(2):
                nc.vector.tensor_mul(
                    out=junk[:, :], in0=s_all[:, :],
                    in1=w_sb[:, chn * GP:(chn + 1) * GP])
                nc.vector.reduce_sum(
                    out=o_sb[:, chn:chn + 1], in_=junk[:, :],
                    axis=mybir.AxisListType.X)
            nc.sync.dma_start(out=o_d, in_=o_sb[:, :])

    return nc


def _get_nc():
    if "nc" not in _cache:
        _cache["nc"] = _build_nc()
    return _cache["nc"]


def _shard_mat(m):
    """(N,16,16) -> (CORES, T, 128, S*256), zero-padded, group-major free axis."""
    out = np.zeros((NP, 256), np.float32)
    out[:N] = np.asarray(m, np.float32).reshape(N, 256)
    return np.ascontiguousarray(out.reshape(CORES, T, 128, S * 256))


def _shard_vec(v):
    """(N,16) -> (CORES, T, 128, S*16)."""
    out = np.zeros((NP, 16), np.float32)
    out[:N] = np.asarray(v, np.float32).reshape(N, 16)
    return np.ascontiguousarray(out.reshape(CORES, T, 128, S * 16))


def kernel(h0, cw0, mw0, cw1, mw1,
           msg0_r0, msg0_r1, msg0_r2,
           msg1_r0, msg1_r1, msg1_r2,
           w_pred, b_pred):
    from concourse.bass_utils import run_bass_kernel_spmd

    nc = _get_nc()
    if not _cache.get("split_done"):
        import concourse.mybir as mybir
        _split_multiwait(nc, mybir)
        _cache["split_done"] = True

    A = _shard_mat(cw0[0])
    B = _shard_mat(mw0[0])
    # interleave D|E per group: (C,T,128,S,2,256) -> (C,T,128,S*512)
    DE = np.ascontiguousarray(
        np.stack([_shard_mat(cw1[0]).reshape(CORES, T, 128, S, 256),
                  _shard_mat(mw1[0]).reshape(CORES, T, 128, S, 256)],
                 axis=4).reshape(CORES, T, 128, S * 512))
    X = _shard_vec(np.asarray(h0, np.float32)[..., 0])
    M0 = _shard_vec(np.asarray(msg0_r0, np.float32)[..., 0])
    M1 = _shard_vec(np.asarray(msg1_r0, np.float32)[..., 0])

    wp = np.zeros((2, NP), np.float32)
    wp[:, :N] = np.asarray(w_pred, np.float32)
    # (2, CORES, T, 128, S) -> (CORES, 128, 2, T, S) -> (CORES, 128, 2*GP)
    W = np.ascontiguousarray(
        wp.reshape(2, CORES, T, 128, S).transpose(1, 3, 0, 2, 4)
        .reshape(CORES, 128, 2 * GP))

    in_maps = [
        {"a": A[i], "b": B[i], "de": DE[i],
         "x": X[i], "m0": M0[i], "m1": M1[i], "w": W[i]}
        for i in range(CORES)
    ]
    res = run_bass_kernel_spmd(nc, in_maps, list(range(CORES)), trace=TRACE)
    _cache["last_res"] = res
    partial = np.zeros(2, np.float64)
    for i in range(CORES):
        partial += res.results[i]["o"].astype(np.float64).sum(axis=0)
    out = (partial + np.asarray(b_pred, np.float64)).astype(np.float32)
    return out.reshape(1, 2)
